# revision 30
# baseline (speedup 1.0000x reference)
"""Trainium2 Bass kernel: causal multi-head attention (B=4, T=2048, D=1024, H=16).

Sharding: tensor-parallel over heads. Each of the 8 cores handles 2 heads
(a 128-wide slice of the head dimension): it computes q/k/v projections for
its heads, causal attention, and a partial output projection
y_partial = o_local @ wo_local^T (bf16).  The full output is the sum of the
8 partials (reduced on host in f32).

Device dataflow (bf16 matmuls, f32 PSUM accumulation):
  qT,kT = w_loc @ x^T       [128, T]  (PSUM accum over 8 k-subtiles)
  v_nat = x-blk^T @ wv^T    [tk=128, 2*64] natural layout -> va (+ones col)
  S^T   = kT_blk^T qT       [tk=128, tq<=512]  K=64/head, both heads run
                            concurrently on PE row strips 0/64
  E = exp(S^T * scale)      (ACT), lower-triangle keep-mask on diag blocks
  PV: po_h[65, tq] += [v_h|1]^T E_h  rows 0:64 = o_un^T, row 64 = Z
  per qt/h: rr = approx-recip(Z row) (DVE via SBUF), rzb = bcast (GpSimd),
            oT = po * rzb   (fused normalize + PSUM->SBUF copy, bf16)
  y_chunk = oT_chunk^T @ wo bf16 -> DMA to DRAM (1024-wide tiles)

All DRAM tensors are host-prepacked so every DMA line is >=2KB contiguous
per partition.  A short burst of dummy matmuls at t=0 warms the PE HAM clock
gate while the first DMAs are in flight.  Emission order software-pipelines
batches at sub-qt granularity: between attention blocks of batch b we emit
projection units of batch b+1 and output-projection units of already
finished query tiles (including batch b's own), keeping the PE stream dense.
"""

import os
import numpy as np

import concourse.bass as bass
import concourse.bacc as bacc
import concourse.mybir as mybir
from concourse.tile import TileContext
from contextlib import ExitStack

# Problem constants (hardcoded per contract)
B, T, D, H = 4, 2048, 1024, 16
HD = D // H            # 64 head dim
P = 128                # partitions
KO = D // P            # 8 contraction subtiles for projections
TQT = 512              # tq tile width
NBLK = T // P          # 16 tk blocks per batch
NCHUNK = T // TQT      # 4 token chunks per batch
NHL = 2                # heads per core
TT = B * T             # 8192 tokens
NCORES = 8
SCALE = 1.0 / float(np.sqrt(np.float32(HD)))

F32 = mybir.dt.float32
BF16 = mybir.dt.bfloat16


def build_program():
    nc = bacc.Bacc("TRN2", target_bir_lowering=False, num_devices=NCORES)
    # host-prepacked layouts: per-partition lines are contiguous
    xt = nc.dram_tensor("xt", [B * NCHUNK, P, KO, TQT], BF16,
                        kind="ExternalInput")
    wq = nc.dram_tensor("wq", [P, KO, P], BF16, kind="ExternalInput")
    wk = nc.dram_tensor("wk", [P, KO, P], BF16, kind="ExternalInput")
    wv = nc.dram_tensor("wv", [P, KO, P], BF16, kind="ExternalInput")
    wo = nc.dram_tensor("wo", [P, D], BF16, kind="ExternalInput")
    cm = nc.dram_tensor("cmask", [P, NHL, P], BF16, kind="ExternalInput")
    y = nc.dram_tensor("y", [B * NBLK, P, D], BF16, kind="ExternalOutput")

    Exp = mybir.ActivationFunctionType.Exp
    Copy = mybir.ActivationFunctionType.Copy
    Mult = mybir.AluOpType.mult

    with TileContext(nc) as tc, ExitStack() as ctx:
        const = ctx.enter_context(tc.tile_pool(name="const", bufs=1))
        qk_pool = ctx.enter_context(tc.tile_pool(name="qk", bufs=2))
        va_pool = ctx.enter_context(tc.tile_pool(name="va", bufs=2))
        o_pool = ctx.enter_context(tc.tile_pool(name="o", bufs=2))
        xt_pool = ctx.enter_context(tc.tile_pool(name="xtp", bufs=4))
        e_pool = ctx.enter_context(tc.tile_pool(name="e", bufs=4))
        z_pool = ctx.enter_context(tc.tile_pool(name="z", bufs=3))
        y_pool = ctx.enter_context(tc.tile_pool(name="yp", bufs=4))
        psA = ctx.enter_context(tc.tile_pool(name="psA", bufs=2, space="PSUM"))
        psS = ctx.enter_context(tc.tile_pool(name="psS", bufs=2, space="PSUM"))
        psO = ctx.enter_context(tc.tile_pool(name="psO", bufs=2, space="PSUM"))

        # --- PE warm-up: dummy matmuls on scratch keep the PE busy (and the
        # HAM clock-gate warming) while the first DMAs are in flight ---
        scratch = const.tile([P, TQT], BF16, tag="scr")
        nc.vector.memset(scratch, 0.0)
        for i in range(28):
            psw = psA.tile([P, TQT], F32, tag="psA")
            nc.tensor.matmul(psw, scratch[:, 0:P], scratch,
                             start=True, stop=True)

        # --- constants into SBUF ---
        wq_sb = const.tile([P, KO, P], BF16, tag="wq")
        wk_sb = const.tile([P, KO, P], BF16, tag="wk")
        wv_sb = const.tile([P, KO, P], BF16, tag="wv")
        for w_sb, w_d in ((wq_sb, wq), (wk_sb, wk), (wv_sb, wv)):
            nc.sync.dma_start(w_sb, w_d[:])
        wo_sb = const.tile([P, D], BF16, tag="wo")
        nc.sync.dma_start(wo_sb, wo[:])
        cm_sb = const.tile([P, NHL, P], BF16, tag="cm")
        nc.sync.dma_start(cm_sb, cm[:])

        def alloc_tiles(b):
            """Allocate batch b's persistent tile set + prefetch its x."""
            qT = qk_pool.tile([P, T], BF16, tag="qT", name=f"qT{b}")
            kT = qk_pool.tile([P, T], BF16, tag="kT", name=f"kT{b}")
            va = va_pool.tile([P, NBLK * NHL, HD + 1], BF16, tag="va",
                              name=f"va{b}")
            oT = o_pool.tile([P, T], BF16, tag="oT", name=f"oT{b}")
            nc.vector.memset(va[:, :, HD : HD + 1], 1.0)
            xxs = []
            for cc in range(NCHUNK):
                xx = xt_pool.tile([P, KO, TQT], BF16, tag="xt",
                                  name=f"xt{b}_{cc}")
                nc.sync.dma_start(xx, xt[b * NCHUNK + cc])
                xxs.append(xx)
            return qT, kT, va, oT, xxs

        def gen_a(b, tiles):
            """Projection filler units for batch b (yields after each ~1us
            chunk of PE work)."""
            qT, kT, va, oT, xxs = tiles
            for cc in range(NCHUNK):
                xx = xxs[cc]
                for w_sb, dst in ((wq_sb, qT), (wk_sb, kT)):
                    ps = psA.tile([P, TQT], F32, tag="psA")
                    for ko in range(KO):
                        nc.tensor.matmul(
                            ps, w_sb[:, ko, :], xx[:, ko, :],
                            start=(ko == 0), stop=(ko == KO - 1),
                        )
                    nc.scalar.activation(
                        dst[:, cc * TQT : (cc + 1) * TQT], ps, Copy
                    )
                    yield
                # v in natural [token, dim] layout: stationary = x^T block
                for pair in range(2):
                    for tb2 in range(2):
                        tb = pair * 2 + tb2
                        blk = cc * 4 + tb
                        psv = psA.tile([P, NHL, HD], F32, tag="psA")
                        for ko in range(KO):
                            nc.tensor.matmul(
                                psv[:, :, :],
                                xx[:, ko, tb * P : (tb + 1) * P],
                                wv_sb[:, ko, :],
                                start=(ko == 0), stop=(ko == KO - 1),
                            )
                        nc.vector.tensor_copy(
                            va[:, blk * NHL : (blk + 1) * NHL, 0:HD], psv
                        )
                    yield

        def gen_c_qt(b, tiles, qt):
            """Output-projection units for batch b, query tile qt (ready as
            soon as qt_finish(b, qt) has been emitted)."""
            qT, kT, va, oT, xxs = tiles
            for u in range(2):
                for i in range(2):
                    tn = qt * 4 + u * 2 + i
                    yt = y_pool.tile([P, D], BF16, tag="yt")
                    for cc in range(D // TQT):
                        psy = psA.tile([P, TQT], F32, tag="psA")
                        nc.tensor.matmul(
                            psy,
                            oT[:, tn * P : (tn + 1) * P],
                            wo_sb[:, cc * TQT : (cc + 1) * TQT],
                            start=True, stop=True,
                        )
                        nc.vector.tensor_copy(
                            yt[:, cc * TQT : (cc + 1) * TQT], psy
                        )
                    nc.sync.dma_start(y[b * NBLK + tn], yt)
                yield

        def attn_block(b, tiles, qt, kb, nblk, po, nfill=0):
            """One S -> exp -> mask -> PV step for query tile qt, key blk kb."""
            qT, kT, va, oT, xxs = tiles
            tq0 = qt * TQT
            m = kb - tq0 // P  # >=0: diagonal-crossing block
            c0 = P * m if m >= 0 else 0
            ps2 = psS.tile([P, NHL, TQT], F32, tag="ps")
            for h in range(NHL):
                hs = slice(h * HD, (h + 1) * HD)
                nc.tensor.matmul(
                    ps2[:, h, c0:TQT],
                    kT[hs, kb * P : (kb + 1) * P],
                    qT[hs, tq0 + c0 : tq0 + TQT],
                    start=True, stop=True,
                )
            et2 = e_pool.tile([P, NHL, TQT], BF16, tag="et")
            nc.scalar.activation(
                et2[:, :, c0:TQT], ps2[:, :, c0:TQT], Exp, scale=SCALE
            )
            if m >= 0:
                # diag block: same lower-triangle keep-mask for every shift
                nc.vector.tensor_tensor(
                    et2[:, :, c0 : c0 + P],
                    et2[:, :, c0 : c0 + P],
                    cm_sb,
                    Mult,
                )
            for _ in range(nfill):
                next_filler()
            for h in range(NHL):
                nc.tensor.matmul(
                    po[h][:, c0:TQT],
                    va[:, kb * NHL + h, :],
                    et2[:, h, c0:TQT],
                    start=(kb == 0), stop=(kb == nblk - 1),
                )

        def qt_finish(b, tiles, qt, po):
            """Normalize: oT[:, qt] = po / Z, fused into the PSUM->SBUF copy."""
            qT, kT, va, oT, xxs = tiles
            tq0 = qt * TQT
            for h in range(NHL):
                hs = slice(h * HD, (h + 1) * HD)
                zr = z_pool.tile([1, TQT], F32, tag="zr")
                if (qt + h) % 2 == 0:
                    nc.vector.tensor_copy(zr, po[h][HD : HD + 1, :])
                else:
                    nc.scalar.activation(zr, po[h][HD : HD + 1, :], Copy)
                rr = z_pool.tile([1, TQT], F32, tag="rr")
                nc.vector.reciprocal_approx_fast(rr, zr)
                rzb = z_pool.tile([HD, TQT], F32, tag="rzb")
                nc.gpsimd.partition_broadcast(rzb, rr)
                nc.vector.tensor_tensor(
                    oT[hs, tq0 : tq0 + TQT],
                    po[h][0:HD, :],
                    rzb,
                    Mult,
                )

        # ---- main software-pipelined schedule ----
        tiles = {0: alloc_tiles(0)}
        # prologue: batch 0 projections emitted densely
        for _ in gen_a(0, tiles[0]):
            pass

        fillers = []

        def next_filler():
            while fillers:
                try:
                    next(fillers[0])
                    return True
                except StopIteration:
                    fillers.pop(0)
            return False

        for b in range(B):
            a_gen = None
            if b + 1 < B:
                tiles[b + 1] = alloc_tiles(b + 1)
                a_gen = gen_a(b + 1, tiles[b + 1])
                fillers.append(a_gen)

            it = 0
            for qt in range(T // TQT):
                tq0 = qt * TQT
                nblk = tq0 // P + TQT // P
                po = [
                    psO.tile([HD + 1, TQT], F32, tag="po", name=f"po{qt%2}_{h}")
                    for h in range(NHL)
                ]
                for kb in range(nblk):
                    it += 1
                    nfill = 0
                    if it % 5 == 0 and it < 40:
                        # last batch has only 8 filler units: spread them
                        nfill = 3 if b + 1 < B else 1
                    attn_block(b, tiles[b], qt, kb, nblk, po, nfill)
                qt_finish(b, tiles[b], qt, po)
                if qt == T // TQT - 1:
                    # batch b+1's projections must all be emitted before its
                    # attention starts
                    while a_gen is not None and a_gen in fillers:
                        next_filler()
                fillers.append(gen_c_qt(b, tiles[b], qt))
                if qt == T // TQT - 1:
                    # reserved filler bridges the batch transition while the
                    # last query tile's normalize chain drains
                    for _ in range(6):
                        next_filler()
            if b > 0:
                del tiles[b - 1]

        # epilogue: drain remaining output-projection units
        while next_filler():
            pass
        del tiles[B - 1]

    nc.compile()
    return nc


def make_core_inputs(x, wq, wk, wv, wo):
    """Host-side sharding/layout prep. Returns list of 8 in_maps."""
    b16 = mybir.dt.np(BF16)
    x = np.asarray(x, dtype=np.float32)
    wq = np.asarray(wq, dtype=np.float32).astype(b16)
    wk = np.asarray(wk, dtype=np.float32).astype(b16)
    wv = np.asarray(wv, dtype=np.float32).astype(b16)
    wo = np.asarray(wo, dtype=np.float32).astype(b16)

    # xt packed [chunk, p, ko*tq]: contraction dk = ko*128 + p
    xt = np.ascontiguousarray(
        np.ascontiguousarray(x.reshape(TT, D).T)  # [D, TT]
        .reshape(KO, P, B * NCHUNK, TQT)
        .transpose(2, 1, 0, 3)
    ).astype(b16)  # [chunk, p, ko, tq]
    # lower-triangle keep-mask for diagonal-crossing blocks, both heads
    i = np.arange(P)[:, None]
    j = np.arange(P)[None, :]
    cmask = np.ascontiguousarray(
        np.broadcast_to((i <= j).astype(np.float32)[:, None, :], (P, NHL, P))
    ).astype(b16)

    def pack_w(w):  # [D, 128] -> [128p, 8ko, 128d]
        return np.ascontiguousarray(w.reshape(KO, P, P).transpose(1, 0, 2))

    in_maps = []
    for c in range(NCORES):
        dr = slice(c * P, (c + 1) * P)
        in_maps.append(
            {
                "xt": xt,
                "wq": pack_w(wq[dr, :].T.copy()),
                "wk": pack_w(wk[dr, :].T.copy()),
                "wv": pack_w(wv[dr, :].T.copy()),
                "wo": np.ascontiguousarray(wo[:, dr].T),
                "cmask": cmask,
            }
        )
    return in_maps


_CACHE = {}


def run(in_maps, **kwargs):
    from concourse.bass_utils import run_bass_kernel_spmd

    if "nc" not in _CACHE:
        _CACHE["nc"] = build_program()
    nc = _CACHE["nc"]
    res = run_bass_kernel_spmd(nc, in_maps, core_ids=list(range(NCORES)), **kwargs)
    return res


def kernel(x, wq, wk, wv, wo):
    in_maps = make_core_inputs(x, wq, wk, wv, wo)
    res = run(in_maps)
    y = np.zeros((TT, D), dtype=np.float32)
    for r in res.results:
        y += np.asarray(r["y"], dtype=np.float32).reshape(TT, D)
    return y.reshape(B, T, D)
